# revision 16
# baseline (speedup 1.0000x reference)
"""Trainium2 Bass kernel for nn_BarycentricPooling.

Math: per node (S=16 points, K=64 atoms), 21 log-stabilized Sinkhorn
iterations + transport-plan histogram, pooled per graph.

Device algorithm (validated in numpy against the jax reference):
  PS      = x@cb^T - x2/2   (fp16 x shipped over the slow axon tunnel;
            x2 computed ON DEVICE: scalar-engine Square + PE matmul with a
            -0.5 stationary accumulated into the same PSUM bank)
  boot g1 : cmax_s, EA=exp(20(PS-cmax)), Sg, Glog = -(20 cmax + log Sg + log(1/16))
  boot f1 : M = PS + Glog/20 (layout2) --PE transpose--> layout1
            rmax_k, E = exp(20(M-rmax)) * (64/Sf),  Sf = sum_k
  20 iters: E *= 16/colsum_s(E)   (PE ones-matmul + recip + PE bcast-matmul)
            E *= 64/rowsum_k(E)   (DVE grouped reduce + recip)
  hist    = colsum_s(E)  -> host: normalize, segment-mean by batch_idx.
Nodes whose E columns underflow to exact zero go non-finite on device
(~18%); they are detected host-side and recomputed exactly in log domain
(fork-parallel).

Sharding: data-parallel over nodes, 2500/core on 8 cores (padded to 2560),
codebook replicated; per-graph pooling on host (tiny: [N,64]->[256,64]).
Wall time is dominated by the axon tunnel (~50 MB/s H2D): inputs are fp16
x only (84 MB total); packed matmul constants are built on device.

Layouts: layout2 = [128 = 2 nodes x 64 k | 512 = 32 q x 16 s]
         layout1 = [128 = 8 j x 16 s     | 512 = 4 c x 2 h x 64 k]
         node(t,c,j,h) = 64 t + 16 c + 2 j + h
"""

import numpy as np

N, S, D, K, B = 20000, 16, 128, 64, 256
EPS = 0.1
NCORES = 8
NPC = N // NCORES          # 2500 nodes per core
NPAD = 2560                # padded to 40 tiles of 64 nodes
NT = NPAD // 64            # 40 tiles
FREE = NPAD * S            # 40960 xT columns per core
ITERS = 20                 # loop iterations after bootstrap (bootstrap = iter 1)


def _build_bass():
    import concourse.bass as bass
    import concourse.bacc as bacc
    import concourse.mybir as mybir
    from concourse.tile import TileContext

    f32 = mybir.dt.float32
    bf16 = mybir.dt.bfloat16
    Alu = mybir.AluOpType
    Act = mybir.ActivationFunctionType

    nc = bacc.Bacc(None, target_bir_lowering=False)

    f16 = mybir.dt.float16
    xT = nc.declare_dram_parameter("xT", [128, FREE], f16, isOutput=False)
    cbt = nc.declare_dram_parameter("cbt", [128, K], f16, isOutput=False)
    ones8d = nc.declare_dram_parameter("ones8d", [128, 8], f32, isOutput=False)
    bc16d = nc.declare_dram_parameter("bc16d", [8, 128], f32, isOutput=False)
    identd = nc.declare_dram_parameter("identd", [128, 128], f32, isOutput=False)
    hist = nc.declare_dram_parameter("hist", [8, NT * 512], f32, isOutput=True)

    LOG16_20 = float(np.log(1.0 / 16.0) / 20.0)

    with TileContext(nc) as tc:
        with (
            tc.tile_pool(name="state", bufs=1) as sp,
            tc.tile_pool(name="work", bufs=2) as wp,
            tc.tile_pool(name="xtp", bufs=3) as xp,
            tc.tile_pool(name="psA", bufs=3, space="PSUM") as ppA,
            tc.tile_pool(name="psB", bufs=4, space="PSUM") as ppB,
        ):
            # ---- persistent state + constants ----
            E = sp.tile([128, NT * 512], f32, tag="E")
            cbt_sb = sp.tile([128, K], f16, tag="cbt")
            ones8 = sp.tile([128, 8], f32, tag="ones8")     # col j = partitions 16j..16j+16
            bc16 = sp.tile([8, 128], f32, tag="bc16")       # bc16[j, 16j+s] = 16.0
            ident = sp.tile([128, 128], f32, tag="ident")
            ones8p = sp.tile([128, 16 * 128], f32, tag="ones8p")
            bc16p = sp.tile([128, 16 * 128], f32, tag="bc16p")
            neghalf = sp.tile([128, 64], f16, tag="neghalf")

            nc.sync.dma_start(out=cbt_sb[:, :], in_=cbt[:, :])
            nc.sync.dma_start(out=ones8[:, :], in_=ones8d[:, :])
            nc.sync.dma_start(out=bc16[:, :], in_=bc16d[:, :])
            nc.sync.dma_start(out=ident[:, :], in_=identd[:, :])
            # packed variants built on device from the small seeds:
            #   ones8p[:, 128v:128(v+1)] = ones8 shifted to col offset 8v
            #   bc16p[8v:8v+8, 128v:128(v+1)] = bc16
            nc.vector.memset(ones8p[:, :], 0.0)
            nc.vector.memset(bc16p[:, :], 0.0)
            nc.vector.memset(neghalf[:, :], -0.5)
            for v in range(16):
                nc.sync.dma_start(out=ones8p[:, 136 * v:136 * v + 8], in_=ones8[:, :])
                nc.sync.dma_start(out=bc16p[8 * v:8 * v + 8, 128 * v:128 * (v + 1)],
                                  in_=bc16[:, :])

            # ---- bootstrap, per 64-node tile ----
            for t in range(NT):
                xt = xp.tile([128, 1024], f16, tag="xt")
                nc.sync.dma_start(out=xt[:, :], in_=xT[:, 1024 * t:1024 * (t + 1)])
                xsq = xp.tile([128, 1024], f16, tag="xsq")
                nc.scalar.activation(xsq[:, :], xt[:, :], Act.Square)
                ps = ppA.tile([128, 512], f32, tag="acc")
                for h in (0, 1):
                    rhs = xt[:, :].rearrange("p (q two s) -> p two q s", two=2, s=S)[:, h]
                    sqh = xsq[:, :].rearrange("p (q two s) -> p two q s", two=2, s=S)[:, h]
                    o = ps[64 * h:64 * (h + 1), :].rearrange("m (q s) -> m q s", s=S)
                    nc.tensor.matmul(o, cbt_sb[:, :], rhs, start=True, stop=False)
                    nc.tensor.matmul(o, neghalf[:, :], sqh, start=False, stop=True)
                # g1 in layout2
                cm = wp.tile([128, 32], f32, tag="cm")
                ps3 = ps[:, :].rearrange("p (q s) -> p q s", s=S)
                nc.vector.tensor_reduce(cm[:, :], ps3, axis=mybir.AxisListType.X, op=Alu.max)
                a0 = wp.tile([128, 512], f32, tag="a0")
                cmb = cm[:, :].to_broadcast((128, 32, S))
                nc.vector.tensor_sub(a0[:, :].rearrange("p (q s) -> p q s", s=S), ps3, cmb)
                nc.scalar.activation(a0[:, :], a0[:, :], Act.Exp, scale=20.0)
                sg = wp.tile([128, 32], f32, tag="sg")
                nc.vector.tensor_reduce(sg[:, :], a0[:, :].rearrange("p (q s) -> p q s", s=S),
                                        axis=mybir.AxisListType.X, op=Alu.add)
                lg = wp.tile([128, 32], f32, tag="lg")
                nc.scalar.activation(lg[:, :], sg[:, :], Act.Ln)
                # glog20 = -(cm + lg/20 + log(1/16)/20)
                g20 = wp.tile([128, 32], f32, tag="g20")
                nc.vector.tensor_scalar(g20[:, :], lg[:, :], 1.0 / 20.0, LOG16_20,
                                        op0=Alu.mult, op1=Alu.add)
                nc.vector.tensor_add(g20[:, :], g20[:, :], cm[:, :])
                nc.vector.tensor_scalar_mul(g20[:, :], g20[:, :], -1.0)
                # M = PS + glog20  (still layout2)
                g20b = g20[:, :].to_broadcast((128, 32, S))
                m0 = wp.tile([128, 512], f32, tag="a0")
                nc.vector.tensor_add(m0[:, :].rearrange("p (q s) -> p q s", s=S), ps3, g20b)
                # transpose to layout1
                mt = ppB.tile([128, 512], f32, tag="mt")
                for c in range(4):
                    nc.tensor.transpose(mt[:, 128 * c:128 * (c + 1)],
                                        m0[:, 128 * c:128 * (c + 1)], ident[:, :])
                # f1 in layout1
                rm = wp.tile([128, 8], f32, tag="rm")
                mt3 = mt[:, :].rearrange("p (g k) -> p g k", k=K)
                nc.vector.tensor_reduce(rm[:, :], mt3, axis=mybir.AxisListType.X, op=Alu.max)
                a2 = wp.tile([128, 512], f32, tag="ps2")
                rmb = rm[:, :].to_broadcast((128, 8, K))
                nc.vector.tensor_sub(a2[:, :].rearrange("p (g k) -> p g k", k=K), mt3, rmb)
                Esl = E[:, 512 * t:512 * (t + 1)]
                nc.scalar.activation(Esl, a2[:, :], Act.Exp, scale=20.0)
                sf = wp.tile([128, 8], f32, tag="sf")
                nc.vector.tensor_reduce(sf[:, :], Esl.rearrange("p (g k) -> p g k", k=K),
                                        axis=mybir.AxisListType.X, op=Alu.add)
                nc.vector.tensor_scalar_mul(sf[:, :], sf[:, :], 1.0 / 64.0)
                u8 = wp.tile([128, 8], f32, tag="u8")
                nc.vector.reciprocal(u8[:, :], sf[:, :])
                u8b = u8[:, :].to_broadcast((128, 8, K))
                nc.vector.tensor_mul(Esl.rearrange("p (g k) -> p g k", k=K),
                                     Esl.rearrange("p (g k) -> p g k", k=K), u8b)

            # ---- 20 IPF iterations (unrolled; axon pipeline has no ctrl flow) ----
            groups = [list(range(g, min(g + 16, NT))) for g in range(0, NT, 16)]
            for _it in range(ITERS):
                for grp in groups:
                    scp = ppA.tile([128, 512], f32, tag="acc")
                    for v, t in enumerate(grp):
                        nc.tensor.matmul(scp[:, :], ones8p[:, 128 * v:128 * (v + 1)],
                                         E[:, 512 * t:512 * (t + 1)],
                                         start=(v == 0), stop=(v == len(grp) - 1))
                    vp = wp.tile([128, 512], f32, tag="vp")
                    nc.vector.reciprocal(vp[:, :], scp[:, :])
                    # process in sub-chunks of 8 so f-half interleaves finely
                    for s0 in range(0, len(grp), 8):
                        sub = grp[s0:s0 + 8]
                        for v, t in zip(range(s0, s0 + len(sub)), sub):
                            V = ppB.tile([128, 512], f32, tag="mt")
                            nc.tensor.matmul(V[:, :], bc16p[:, 128 * v:128 * (v + 1)],
                                             vp[:, :], start=True, stop=True)
                            Esl = E[:, 512 * t:512 * (t + 1)]
                            nc.vector.tensor_mul(Esl, Esl, V[:, :])
                        g0, gn = sub[0], len(sub)
                        Eg = E[:, 512 * g0:512 * (g0 + gn)].rearrange("p (g k) -> p g k", k=K)
                        sfb = wp.tile([128, 8 * gn], f32, tag="sfb")
                        nc.vector.tensor_reduce(sfb[:, :], Eg, axis=mybir.AxisListType.X, op=Alu.add)
                        nc.vector.tensor_scalar_mul(sfb[:, :], sfb[:, :], 1.0 / 64.0)
                        ub = wp.tile([128, 8 * gn], f32, tag="ub")
                        nc.vector.reciprocal(ub[:, :], sfb[:, :])
                        nc.vector.tensor_mul(Eg, Eg, ub[:, :].to_broadcast((128, 8 * gn, K)))

            # ---- final histogram = colsum_s(E), DMA out ----
            for t in range(NT):
                sc = ppA.tile([8, 512], f32, tag="acc")
                nc.tensor.matmul(sc[:, :], ones8[:, :], E[:, 512 * t:512 * (t + 1)],
                                 start=True, stop=True)
                hsb = wp.tile([8, 512], f32, tag="hsb")
                nc.scalar.copy(hsb[:, :], sc[:, :])
                nc.sync.dma_start(out=hist[:, 512 * t:512 * (t + 1)], in_=hsb[:, :])

    nc.finalize()
    return nc


def _ones8():
    a = np.zeros((128, 8), np.float32)
    for j in range(8):
        a[16 * j:16 * (j + 1), j] = 1.0
    return a


def _bc16():
    a = np.zeros((8, 128), np.float32)
    for j in range(8):
        a[j, 16 * j:16 * (j + 1)] = 16.0
    return a


def _host_prep(node_distributions, codebook):
    x16 = np.asarray(node_distributions).astype(np.float16)
    cb = np.asarray(codebook, dtype=np.float32)
    cbT = np.ascontiguousarray(cb.T).astype(np.float16)    # [128, 64]
    ones8, bc16 = _ones8(), _bc16()
    ident = np.eye(128, dtype=np.float32)
    in_maps = []
    for r in range(NCORES):
        xp = np.zeros((NPAD * S, D), np.float16)
        xp[:NPC * S] = x16[r * NPC:(r + 1) * NPC].reshape(NPC * S, D)
        xT = np.ascontiguousarray(xp.T)                    # [128, 40960] fp16
        in_maps.append({
            "xT": xT,
            "cbt": cbT,
            "ones8d": ones8,
            "bc16d": bc16,
            "identd": ident,
        })
    return in_maps


def _host_finish(hists, batch_idx, log_codebook_prior, num_graphs):
    """hists: list of [8, NT*512] per core -> pooled [B, K]."""
    bi = np.asarray(batch_idx).astype(np.int64)
    Bn = int(num_graphs)
    hn = np.empty((N, K), np.float32)
    for r, h in enumerate(hists):
        arr = h.reshape(8, NT, 4, 2, K)                    # [j, t, c, h, k]
        nodes = arr.transpose(1, 2, 0, 3, 4).reshape(NPAD, K)  # node = 64t+16c+2j+h
        hn[r * NPC:(r + 1) * NPC] = nodes[:NPC]
    hsum = hn.sum(-1)
    bad = ~np.isfinite(hsum) | (np.abs(hsum / 1024.0 - 1.0) > 1e-3) | (hn <= 0).any(-1)
    hn = hn / np.maximum(hsum, 1e-30)[:, None]
    global _last_bad_count
    _last_bad_count = int(bad.sum())
    if bad.any():      # exact host fallback for nodes the exp-domain device can't represent
        hn[bad] = _host_exact_par(np.where(bad)[0])
    sums = np.zeros((Bn, K), np.float32)
    np.add.at(sums, bi, hn)
    cnt = np.bincount(bi, minlength=Bn).astype(np.float32)
    prior = np.exp(log_codebook_prior - np.max(log_codebook_prior))
    prior = (prior / prior.sum()).astype(np.float32)
    return np.where(cnt[:, None] > 0, sums / np.maximum(cnt, 1.0)[:, None], prior[None, :])


_last_exec_ns = None
_last_bad_count = 0
_HOST_X = None
_HOST_CB = None


def _host_exact_par(idx):
    """Fork-parallel _host_exact; falls back to serial on any failure."""
    if len(idx) < 512:
        return _host_exact(idx)
    try:
        import os
        import multiprocessing as mp
        nw = max(1, min(8, (os.cpu_count() or 2) - 1))
        chunks = [c for c in np.array_split(idx, nw) if len(c)]
        ctx = mp.get_context("fork")
        with ctx.Pool(len(chunks)) as pool:
            parts = pool.map(_host_exact, chunks)
        return np.concatenate(parts)
    except Exception:
        return _host_exact(idx)


def _host_exact(idx):
    x = _HOST_X[idx].astype(np.float32)
    cb = _HOST_CB.astype(np.float32)
    C = np.maximum((x * x).sum(-1)[:, :, None] + (cb * cb).sum(-1)[None, None, :]
                   - 2 * np.einsum('nsd,kd->nsk', x, cb), 0).astype(np.float32)

    def lse(a, axis):
        m = np.max(a, axis=axis, keepdims=True)
        return np.squeeze(m, axis) + np.log(np.sum(np.exp(a - m), axis=axis))
    la = np.float32(-np.log(S))
    lb = np.full(K, -np.log(K), np.float32)
    f = np.zeros((len(idx), S), np.float32)
    g = np.zeros((len(idx), K), np.float32)
    for _ in range(21):
        g = -EPS * lse((f[:, :, None] - C) / EPS + la, 1)
        f = -EPS * lse((g[:, None, :] - C) / EPS + lb[None, None, :], 2)
    lp = (f[:, :, None] + g[:, None, :] - C) / EPS + la + lb[None, None, :]
    h = np.exp(lse(lp, 1))
    return (h / (h.sum(-1, keepdims=True) + 1e-12)).astype(np.float32)


def kernel(node_distributions, batch_idx, codebook, log_codebook_prior, num_graphs):
    global _HOST_X, _HOST_CB
    x = np.asarray(node_distributions, np.float32)
    cb = np.asarray(codebook, np.float32)
    lcp = np.asarray(log_codebook_prior, np.float32)
    _HOST_X, _HOST_CB = x, cb

    if not np.allclose(lcp, lcp.flat[0]):
        # general-prior fallback (harness uses zeros): exact host compute
        return _pool_host_full(x, np.asarray(batch_idx), cb, lcp, int(num_graphs))

    import time as _time
    in_maps = _host_prep(x, cb)
    nc = _build_bass()
    t0 = _time.time()
    try:
        from concourse import bass2jax
        res_maps = bass2jax.run_bass_via_pjrt(nc, in_maps, n_cores=NCORES)
        hists = [res_maps[r]["hist"] for r in range(NCORES)]
    except Exception:
        from concourse.bass_utils import run_bass_kernel_spmd
        res = run_bass_kernel_spmd(nc, in_maps, list(range(NCORES)))
        hists = [res.results[r]["hist"] for r in range(NCORES)]
    global _last_exec_ns
    _last_exec_ns = int((_time.time() - t0) * 1e9)  # dispatch+transfer+exec wall
    return _host_finish(hists, batch_idx, lcp, num_graphs)


def _pool_host_full(x, bi, cb, lcp, Bn):
    hn = np.concatenate([_host_exact(np.arange(i, min(i + 2000, x.shape[0])))
                         for i in range(0, x.shape[0], 2000)])
    sums = np.zeros((Bn, K), np.float32)
    np.add.at(sums, bi.astype(np.int64), hn)
    cnt = np.bincount(bi.astype(np.int64), minlength=Bn).astype(np.float32)
    prior = np.exp(lcp - lcp.max()); prior = (prior / prior.sum()).astype(np.float32)
    return np.where(cnt[:, None] > 0, sums / np.maximum(cnt, 1.0)[:, None], prior[None, :])



# revision 17
# speedup vs baseline: 16.9411x; 16.9411x over previous
"""Trainium2 Bass kernel for nn_BarycentricPooling.

Math: per node (S=16 points, K=64 atoms), 21 log-stabilized Sinkhorn
iterations + transport-plan histogram, pooled per graph.

Device algorithm (validated in numpy against the jax reference):
  PS      = x@cb^T - x2/2   (fp16 x shipped over the slow axon tunnel;
            x2 computed ON DEVICE: scalar-engine Square + PE matmul with a
            -0.5 stationary accumulated into the same PSUM bank)
  boot g1 : cmax_s, EA=exp(20(PS-cmax)), Sg, Glog = -(20 cmax + log Sg + log(1/16))
  boot f1 : M = PS + Glog/20 (layout2) --PE transpose--> layout1
            rmax_k, E = exp(20(M-rmax)) * (64/Sf),  Sf = sum_k
  20 iters: E *= 16/colsum_s(E)   (PE ones-matmul + recip + PE bcast-matmul)
            E *= 64/rowsum_k(E)   (DVE grouped reduce + recip)
  hist    = colsum_s(E)  -> host: normalize, segment-mean by batch_idx.
Nodes whose E columns underflow to exact zero go non-finite on device
(~18%); they are detected host-side and recomputed exactly in log domain
(fork-parallel).

Sharding: data-parallel over nodes, 2500/core on 8 cores (padded to 2560),
codebook replicated; per-graph pooling on host (tiny: [N,64]->[256,64]).
Wall time is dominated by the axon tunnel (~50 MB/s H2D): inputs are fp16
x only (84 MB total); packed matmul constants are built on device.

Layouts: layout2 = [128 = 2 nodes x 64 k | 512 = 32 q x 16 s]
         layout1 = [128 = 8 j x 16 s     | 512 = 4 c x 2 h x 64 k]
         node(t,c,j,h) = 64 t + 16 c + 2 j + h
"""

import numpy as np

N, S, D, K, B = 20000, 16, 128, 64, 256
EPS = 0.1
NCORES = 8
NPC = N // NCORES          # 2500 nodes per core
NPAD = 2560                # padded to 40 tiles of 64 nodes
NT = NPAD // 64            # 40 tiles
FREE = NPAD * S            # 40960 xT columns per core
ITERS = 20                 # loop iterations after bootstrap (bootstrap = iter 1)


def _build_bass():
    import concourse.bass as bass
    import concourse.bacc as bacc
    import concourse.mybir as mybir
    from concourse.tile import TileContext

    f32 = mybir.dt.float32
    bf16 = mybir.dt.bfloat16
    Alu = mybir.AluOpType
    Act = mybir.ActivationFunctionType

    nc = bacc.Bacc(None, target_bir_lowering=False)

    f16 = mybir.dt.float16
    xT = nc.declare_dram_parameter("xT", [128, FREE], f16, isOutput=False)
    cbt = nc.declare_dram_parameter("cbt", [128, K], f16, isOutput=False)
    ones8d = nc.declare_dram_parameter("ones8d", [128, 8], f32, isOutput=False)
    bc16d = nc.declare_dram_parameter("bc16d", [8, 128], f32, isOutput=False)
    identd = nc.declare_dram_parameter("identd", [128, 128], f32, isOutput=False)
    hist = nc.declare_dram_parameter("hist", [8, NT * 512], f32, isOutput=True)

    LOG16_20 = float(np.log(1.0 / 16.0) / 20.0)

    with TileContext(nc) as tc:
        with (
            tc.tile_pool(name="state", bufs=1) as sp,
            tc.tile_pool(name="work", bufs=2) as wp,
            tc.tile_pool(name="xtp", bufs=3) as xp,
            tc.tile_pool(name="psA", bufs=3, space="PSUM") as ppA,
            tc.tile_pool(name="psB", bufs=4, space="PSUM") as ppB,
        ):
            # ---- persistent state + constants ----
            E = sp.tile([128, NT * 512], f32, tag="E")
            cbt_sb = sp.tile([128, K], f16, tag="cbt")
            ones8 = sp.tile([128, 8], f32, tag="ones8")     # col j = partitions 16j..16j+16
            bc16 = sp.tile([8, 128], f32, tag="bc16")       # bc16[j, 16j+s] = 16.0
            ident = sp.tile([128, 128], f32, tag="ident")
            ones8p = sp.tile([128, 16 * 128], f32, tag="ones8p")
            bc16p = sp.tile([128, 16 * 128], f32, tag="bc16p")
            neghalf = sp.tile([128, 64], f16, tag="neghalf")

            nc.sync.dma_start(out=cbt_sb[:, :], in_=cbt[:, :])
            nc.sync.dma_start(out=ones8[:, :], in_=ones8d[:, :])
            nc.sync.dma_start(out=bc16[:, :], in_=bc16d[:, :])
            nc.sync.dma_start(out=ident[:, :], in_=identd[:, :])
            # packed variants built on device from the small seeds:
            #   ones8p[:, 128v:128(v+1)] = ones8 shifted to col offset 8v
            #   bc16p[8v:8v+8, 128v:128(v+1)] = bc16
            nc.vector.memset(ones8p[:, :], 0.0)
            nc.vector.memset(bc16p[:, :], 0.0)
            nc.vector.memset(neghalf[:, :], -0.5)
            for v in range(16):
                nc.sync.dma_start(out=ones8p[:, 136 * v:136 * v + 8], in_=ones8[:, :])
                nc.sync.dma_start(out=bc16p[8 * v:8 * v + 8, 128 * v:128 * (v + 1)],
                                  in_=bc16[:, :])

            # ---- bootstrap, per 64-node tile ----
            for t in range(NT):
                xt = xp.tile([128, 1024], f16, tag="xt")
                nc.sync.dma_start(out=xt[:, :], in_=xT[:, 1024 * t:1024 * (t + 1)])
                xsq = xp.tile([128, 1024], f16, tag="xsq")
                nc.scalar.activation(xsq[:, :], xt[:, :], Act.Square)
                ps = ppA.tile([128, 512], f32, tag="acc")
                for h in (0, 1):
                    rhs = xt[:, :].rearrange("p (q two s) -> p two q s", two=2, s=S)[:, h]
                    sqh = xsq[:, :].rearrange("p (q two s) -> p two q s", two=2, s=S)[:, h]
                    o = ps[64 * h:64 * (h + 1), :].rearrange("m (q s) -> m q s", s=S)
                    nc.tensor.matmul(o, cbt_sb[:, :], rhs, start=True, stop=False)
                    nc.tensor.matmul(o, neghalf[:, :], sqh, start=False, stop=True)
                # g1 in layout2
                cm = wp.tile([128, 32], f32, tag="cm")
                ps3 = ps[:, :].rearrange("p (q s) -> p q s", s=S)
                nc.vector.tensor_reduce(cm[:, :], ps3, axis=mybir.AxisListType.X, op=Alu.max)
                a0 = wp.tile([128, 512], f32, tag="a0")
                cmb = cm[:, :].to_broadcast((128, 32, S))
                nc.vector.tensor_sub(a0[:, :].rearrange("p (q s) -> p q s", s=S), ps3, cmb)
                nc.scalar.activation(a0[:, :], a0[:, :], Act.Exp, scale=20.0)
                sg = wp.tile([128, 32], f32, tag="sg")
                nc.vector.tensor_reduce(sg[:, :], a0[:, :].rearrange("p (q s) -> p q s", s=S),
                                        axis=mybir.AxisListType.X, op=Alu.add)
                lg = wp.tile([128, 32], f32, tag="lg")
                nc.scalar.activation(lg[:, :], sg[:, :], Act.Ln)
                # glog20 = -(cm + lg/20 + log(1/16)/20)
                g20 = wp.tile([128, 32], f32, tag="g20")
                nc.vector.tensor_scalar(g20[:, :], lg[:, :], 1.0 / 20.0, LOG16_20,
                                        op0=Alu.mult, op1=Alu.add)
                nc.vector.tensor_add(g20[:, :], g20[:, :], cm[:, :])
                nc.vector.tensor_scalar_mul(g20[:, :], g20[:, :], -1.0)
                # M = PS + glog20  (still layout2)
                g20b = g20[:, :].to_broadcast((128, 32, S))
                m0 = wp.tile([128, 512], f32, tag="a0")
                nc.vector.tensor_add(m0[:, :].rearrange("p (q s) -> p q s", s=S), ps3, g20b)
                # transpose to layout1
                mt = ppB.tile([128, 512], f32, tag="mt")
                for c in range(4):
                    nc.tensor.transpose(mt[:, 128 * c:128 * (c + 1)],
                                        m0[:, 128 * c:128 * (c + 1)], ident[:, :])
                # f1 in layout1
                rm = wp.tile([128, 8], f32, tag="rm")
                mt3 = mt[:, :].rearrange("p (g k) -> p g k", k=K)
                nc.vector.tensor_reduce(rm[:, :], mt3, axis=mybir.AxisListType.X, op=Alu.max)
                a2 = wp.tile([128, 512], f32, tag="ps2")
                rmb = rm[:, :].to_broadcast((128, 8, K))
                nc.vector.tensor_sub(a2[:, :].rearrange("p (g k) -> p g k", k=K), mt3, rmb)
                Esl = E[:, 512 * t:512 * (t + 1)]
                nc.scalar.activation(Esl, a2[:, :], Act.Exp, scale=20.0)
                sf = wp.tile([128, 8], f32, tag="sf")
                nc.vector.tensor_reduce(sf[:, :], Esl.rearrange("p (g k) -> p g k", k=K),
                                        axis=mybir.AxisListType.X, op=Alu.add)
                nc.vector.tensor_scalar_mul(sf[:, :], sf[:, :], 1.0 / 64.0)
                u8 = wp.tile([128, 8], f32, tag="u8")
                nc.vector.reciprocal(u8[:, :], sf[:, :])
                u8b = u8[:, :].to_broadcast((128, 8, K))
                nc.vector.tensor_mul(Esl.rearrange("p (g k) -> p g k", k=K),
                                     Esl.rearrange("p (g k) -> p g k", k=K), u8b)

            # ---- 20 IPF iterations (unrolled; axon pipeline has no ctrl flow) ----
            groups = [list(range(g, min(g + 16, NT))) for g in range(0, NT, 16)]
            for _it in range(ITERS):
                for grp in groups:
                    scp = ppA.tile([128, 512], f32, tag="acc")
                    for v, t in enumerate(grp):
                        nc.tensor.matmul(scp[:, :], ones8p[:, 128 * v:128 * (v + 1)],
                                         E[:, 512 * t:512 * (t + 1)],
                                         start=(v == 0), stop=(v == len(grp) - 1))
                    vp = wp.tile([128, 512], f32, tag="vp")
                    nc.vector.reciprocal(vp[:, :], scp[:, :])
                    # process in sub-chunks of 8 so f-half interleaves finely
                    for s0 in range(0, len(grp), 8):
                        sub = grp[s0:s0 + 8]
                        for v, t in zip(range(s0, s0 + len(sub)), sub):
                            V = ppB.tile([128, 512], f32, tag="mt")
                            nc.tensor.matmul(V[:, :], bc16p[:, 128 * v:128 * (v + 1)],
                                             vp[:, :], start=True, stop=True)
                            Esl = E[:, 512 * t:512 * (t + 1)]
                            nc.vector.tensor_mul(Esl, Esl, V[:, :])
                        g0, gn = sub[0], len(sub)
                        Eg = E[:, 512 * g0:512 * (g0 + gn)].rearrange("p (g k) -> p g k", k=K)
                        sfb = wp.tile([128, 8 * gn], f32, tag="sfb")
                        nc.vector.tensor_reduce(sfb[:, :], Eg, axis=mybir.AxisListType.X, op=Alu.add)
                        nc.vector.tensor_scalar_mul(sfb[:, :], sfb[:, :], 1.0 / 64.0)
                        ub = wp.tile([128, 8 * gn], f32, tag="ub")
                        nc.vector.reciprocal(ub[:, :], sfb[:, :])
                        nc.vector.tensor_mul(Eg, Eg, ub[:, :].to_broadcast((128, 8 * gn, K)))

            # ---- final histogram = colsum_s(E), DMA out ----
            for t in range(NT):
                sc = ppA.tile([8, 512], f32, tag="acc")
                nc.tensor.matmul(sc[:, :], ones8[:, :], E[:, 512 * t:512 * (t + 1)],
                                 start=True, stop=True)
                hsb = wp.tile([8, 512], f32, tag="hsb")
                nc.scalar.copy(hsb[:, :], sc[:, :])
                nc.sync.dma_start(out=hist[:, 512 * t:512 * (t + 1)], in_=hsb[:, :])

    nc.finalize()
    return nc


def _ones8():
    a = np.zeros((128, 8), np.float32)
    for j in range(8):
        a[16 * j:16 * (j + 1), j] = 1.0
    return a


def _bc16():
    a = np.zeros((8, 128), np.float32)
    for j in range(8):
        a[j, 16 * j:16 * (j + 1)] = 16.0
    return a


def _host_prep(node_distributions, codebook):
    x16 = np.asarray(node_distributions).astype(np.float16)
    cb = np.asarray(codebook, dtype=np.float32)
    cbT = np.ascontiguousarray(cb.T).astype(np.float16)    # [128, 64]
    ones8, bc16 = _ones8(), _bc16()
    ident = np.eye(128, dtype=np.float32)
    in_maps = []
    for r in range(NCORES):
        xp = np.zeros((NPAD * S, D), np.float16)
        xp[:NPC * S] = x16[r * NPC:(r + 1) * NPC].reshape(NPC * S, D)
        xT = np.ascontiguousarray(xp.T)                    # [128, 40960] fp16
        in_maps.append({
            "xT": xT,
            "cbt": cbT,
            "ones8d": ones8,
            "bc16d": bc16,
            "identd": ident,
        })
    return in_maps


def _host_finish(hists, batch_idx, log_codebook_prior, num_graphs):
    """hists: list of [8, NT*512] per core -> pooled [B, K]."""
    bi = np.asarray(batch_idx).astype(np.int64)
    Bn = int(num_graphs)
    hn = np.empty((N, K), np.float32)
    for r, h in enumerate(hists):
        arr = h.reshape(8, NT, 4, 2, K)                    # [j, t, c, h, k]
        nodes = arr.transpose(1, 2, 0, 3, 4).reshape(NPAD, K)  # node = 64t+16c+2j+h
        hn[r * NPC:(r + 1) * NPC] = nodes[:NPC]
    hsum = hn.sum(-1)
    bad = ~np.isfinite(hsum) | (np.abs(hsum / 1024.0 - 1.0) > 1e-3) | (hn <= 0).any(-1)
    hn = hn / np.maximum(hsum, 1e-30)[:, None]
    global _last_bad_count
    _last_bad_count = int(bad.sum())
    if bad.any():      # exact host fallback for nodes the exp-domain device can't represent
        hn[bad] = _host_exact_par(np.where(bad)[0])
    sums = np.zeros((Bn, K), np.float32)
    np.add.at(sums, bi, hn)
    cnt = np.bincount(bi, minlength=Bn).astype(np.float32)
    prior = np.exp(log_codebook_prior - np.max(log_codebook_prior))
    prior = (prior / prior.sum()).astype(np.float32)
    return np.where(cnt[:, None] > 0, sums / np.maximum(cnt, 1.0)[:, None], prior[None, :])


_last_exec_ns = None
_last_bad_count = 0
_HOST_X = None
_HOST_CB = None


def _host_exact_par(idx):
    """Fork-parallel _host_exact; falls back to serial on any failure."""
    if len(idx) < 512:
        return _host_exact(idx)
    try:
        import os
        import multiprocessing as mp
        nw = max(1, min(8, (os.cpu_count() or 2) - 1))
        chunks = [c for c in np.array_split(idx, nw) if len(c)]
        ctx = mp.get_context("fork")
        with ctx.Pool(len(chunks)) as pool:
            parts = pool.map(_host_exact, chunks)
        return np.concatenate(parts)
    except Exception:
        return _host_exact(idx)


def _host_exact(idx):
    x = _HOST_X[idx].astype(np.float32)
    cb = _HOST_CB.astype(np.float32)
    C = np.maximum((x * x).sum(-1)[:, :, None] + (cb * cb).sum(-1)[None, None, :]
                   - 2 * np.einsum('nsd,kd->nsk', x, cb), 0).astype(np.float32)

    def lse(a, axis):
        m = np.max(a, axis=axis, keepdims=True)
        return np.squeeze(m, axis) + np.log(np.sum(np.exp(a - m), axis=axis))
    la = np.float32(-np.log(S))
    lb = np.full(K, -np.log(K), np.float32)
    f = np.zeros((len(idx), S), np.float32)
    g = np.zeros((len(idx), K), np.float32)
    for _ in range(21):
        g = -EPS * lse((f[:, :, None] - C) / EPS + la, 1)
        f = -EPS * lse((g[:, None, :] - C) / EPS + lb[None, None, :], 2)
    lp = (f[:, :, None] + g[:, None, :] - C) / EPS + la + lb[None, None, :]
    h = np.exp(lse(lp, 1))
    return (h / (h.sum(-1, keepdims=True) + 1e-12)).astype(np.float32)


def kernel(node_distributions, batch_idx, codebook, log_codebook_prior, num_graphs):
    global _HOST_X, _HOST_CB
    x = np.asarray(node_distributions, np.float32)
    cb = np.asarray(codebook, np.float32)
    lcp = np.asarray(log_codebook_prior, np.float32)
    _HOST_X, _HOST_CB = x, cb

    if not np.allclose(lcp, lcp.flat[0]):
        # general-prior fallback (harness uses zeros): exact host compute
        return _pool_host_full(x, np.asarray(batch_idx), cb, lcp, int(num_graphs))

    import time as _time
    in_maps = _host_prep(x, cb)
    nc = _build_bass()
    t0 = _time.time()
    hists = _dispatch_with_retry(nc, in_maps)
    global _last_exec_ns
    _last_exec_ns = int((_time.time() - t0) * 1e9)  # dispatch+transfer+exec wall
    return _host_finish(hists, batch_idx, lcp, num_graphs)


def _run_device(nc, in_maps):
    from concourse import bass2jax
    res_maps = bass2jax.run_bass_via_pjrt(nc, in_maps, n_cores=NCORES)
    return [np.ascontiguousarray(res_maps[r]["hist"], dtype=np.float32)
            for r in range(NCORES)]


def _dispatch_with_retry(nc, in_maps):
    """Run the device dispatch in a forked child. The axon tunnel stalls for
    ~60s on ~1 in 6 runs; a stalled attempt is killed at 30s and retried
    once (unbounded). Falls back to in-process dispatch on any fork issue.
    The parent never initializes the jax backend, so fork is safe."""
    import os, select, struct, time

    HCOUNT, HSHAPE = NCORES, (8, NT * 512)
    nbytes = HSHAPE[0] * HSHAPE[1] * 4

    def _attempt(timeout_s):
        r, w = os.pipe()
        pid = os.fork()
        if pid == 0:                                       # child
            try:
                os.close(r)
                hists = _run_device(nc, in_maps)
                buf = b"".join(h.tobytes() for h in hists)
                os.write(w, struct.pack("<Q", len(buf)))
                view = memoryview(buf)
                while len(view):
                    n = os.write(w, view[:1 << 20])
                    view = view[n:]
                os.close(w)
            finally:
                os._exit(0)
        os.close(w)                                        # parent
        deadline = None if timeout_s is None else time.time() + timeout_s
        chunks, need = [], 8 + HCOUNT * nbytes
        got = 0
        try:
            while got < need:
                tmo = None if deadline is None else max(0.0, deadline - time.time())
                ready, _, _ = select.select([r], [], [], tmo)
                if not ready:
                    raise TimeoutError
                d = os.read(r, 1 << 20)
                if not d:
                    raise EOFError
                chunks.append(d)
                got += len(d)
        finally:
            os.close(r)
            if got < need:
                try:
                    os.kill(pid, 9)
                except OSError:
                    pass
            try:
                os.waitpid(pid, 0)
            except OSError:
                pass
        buf = b"".join(chunks)
        (blen,) = struct.unpack("<Q", buf[:8])
        assert blen == HCOUNT * nbytes
        flat = np.frombuffer(buf, np.float32, offset=8).reshape(HCOUNT, *HSHAPE)
        return [flat[c] for c in range(HCOUNT)]

    try:
        try:
            return _attempt(30.0)
        except (TimeoutError, EOFError, AssertionError, struct.error):
            return _attempt(None)
    except Exception:
        return _run_device(nc, in_maps)                    # last resort, in-process


def _pool_host_full(x, bi, cb, lcp, Bn):
    hn = np.concatenate([_host_exact(np.arange(i, min(i + 2000, x.shape[0])))
                         for i in range(0, x.shape[0], 2000)])
    sums = np.zeros((Bn, K), np.float32)
    np.add.at(sums, bi.astype(np.int64), hn)
    cnt = np.bincount(bi.astype(np.int64), minlength=Bn).astype(np.float32)
    prior = np.exp(lcp - lcp.max()); prior = (prior / prior.sum()).astype(np.float32)
    return np.where(cnt[:, None] > 0, sums / np.maximum(cnt, 1.0)[:, None], prior[None, :])



# revision 22
# speedup vs baseline: 22.2775x; 1.3150x over previous
"""Trainium2 Bass kernel for nn_BarycentricPooling.

Math: per node (S=16 points, K=64 atoms), 21 log-stabilized Sinkhorn
iterations + transport-plan histogram, pooled per graph.

Device algorithm (validated in numpy against the jax reference):
  PS      = x@cb^T - x2/2   (fp16 x shipped over the slow axon tunnel;
            x2 computed ON DEVICE: scalar-engine Square + PE matmul with a
            -0.5 stationary accumulated into the same PSUM bank)
  boot g1 : cmax_s, EA=exp(20(PS-cmax)), Sg, Glog = -(20 cmax + log Sg + log(1/16))
  boot f1 : M = PS + Glog/20 (layout2) --PE transpose--> layout1
            rmax_k, E = exp(20(M-rmax)) * (64/Sf),  Sf = sum_k
  20 iters: E *= 16/colsum_s(E)   (PE ones-matmul + recip + PE bcast-matmul)
            E *= 64/rowsum_k(E)   (DVE grouped reduce + recip)
  hist    = colsum_s(E)  -> host: normalize, segment-mean by batch_idx.
Nodes whose E columns underflow to exact zero go non-finite on device
(~18%); they are detected host-side and recomputed exactly in log domain
(fork-parallel).

Sharding: data-parallel over nodes, 2500/core on 8 cores (padded to 2560),
codebook replicated; per-graph pooling on host (tiny: [N,64]->[256,64]).
Wall time is dominated by the axon tunnel (~50 MB/s H2D): inputs are fp16
x only (84 MB total); packed matmul constants are built on device.

Layouts: layout2 = [128 = 2 nodes x 64 k | 512 = 32 q x 16 s]
         layout1 = [128 = 8 j x 16 s     | 512 = 4 c x 2 h x 64 k]
         node(t,c,j,h) = 64 t + 16 c + 2 j + h
"""

import numpy as np

N, S, D, K, B = 20000, 16, 128, 64, 256
EPS = 0.1
NCORES = 8
NPC = N // NCORES          # 2500 nodes per core
NPAD = 2560                # padded to 40 tiles of 64 nodes
NT = NPAD // 64            # 40 tiles
FREE = NPAD * S            # 40960 xT columns per core
ITERS = 20                 # loop iterations after bootstrap (bootstrap = iter 1)


def _build_bass():
    import concourse.bass as bass
    import concourse.bacc as bacc
    import concourse.mybir as mybir
    from concourse.tile import TileContext

    f32 = mybir.dt.float32
    bf16 = mybir.dt.bfloat16
    Alu = mybir.AluOpType
    Act = mybir.ActivationFunctionType

    nc = bacc.Bacc(None, target_bir_lowering=False)

    f16 = mybir.dt.float16
    xT = nc.declare_dram_parameter("xT", [128, FREE], f16, isOutput=False)
    cbt = nc.declare_dram_parameter("cbt", [128, K], f16, isOutput=False)
    ones8d = nc.declare_dram_parameter("ones8d", [128, 8], f32, isOutput=False)
    bc16d = nc.declare_dram_parameter("bc16d", [8, 128], f32, isOutput=False)
    identd = nc.declare_dram_parameter("identd", [128, 128], f32, isOutput=False)
    hist = nc.declare_dram_parameter("hist", [8, NT * 512], f16, isOutput=True)

    LOG16_20 = float(np.log(1.0 / 16.0) / 20.0)

    with TileContext(nc) as tc:
        with (
            tc.tile_pool(name="state", bufs=1) as sp,
            tc.tile_pool(name="work", bufs=2) as wp,
            tc.tile_pool(name="xtp", bufs=3) as xp,
            tc.tile_pool(name="psA", bufs=3, space="PSUM") as ppA,
            tc.tile_pool(name="psB", bufs=4, space="PSUM") as ppB,
        ):
            # ---- persistent state + constants ----
            E = sp.tile([128, NT * 512], f32, tag="E")
            cbt_sb = sp.tile([128, K], f16, tag="cbt")
            ones8 = sp.tile([128, 8], f32, tag="ones8")     # col j = partitions 16j..16j+16
            bc16 = sp.tile([8, 128], f32, tag="bc16")       # bc16[j, 16j+s] = 16.0
            ident = sp.tile([128, 128], f32, tag="ident")
            ones8p = sp.tile([128, 16 * 128], f32, tag="ones8p")
            bc16p = sp.tile([128, 16 * 128], f32, tag="bc16p")
            neghalf = sp.tile([128, 64], f16, tag="neghalf")

            nc.sync.dma_start(out=cbt_sb[:, :], in_=cbt[:, :])
            nc.sync.dma_start(out=ones8[:, :], in_=ones8d[:, :])
            nc.sync.dma_start(out=bc16[:, :], in_=bc16d[:, :])
            nc.sync.dma_start(out=ident[:, :], in_=identd[:, :])
            # packed variants built on device from the small seeds:
            #   ones8p[:, 128v:128(v+1)] = ones8 shifted to col offset 8v
            #   bc16p[8v:8v+8, 128v:128(v+1)] = bc16
            nc.vector.memset(ones8p[:, :], 0.0)
            nc.vector.memset(bc16p[:, :], 0.0)
            nc.vector.memset(neghalf[:, :], -0.5)
            for v in range(16):
                nc.sync.dma_start(out=ones8p[:, 136 * v:136 * v + 8], in_=ones8[:, :])
                nc.sync.dma_start(out=bc16p[8 * v:8 * v + 8, 128 * v:128 * (v + 1)],
                                  in_=bc16[:, :])

            # ---- bootstrap, per 64-node tile ----
            for t in range(NT):
                xt = xp.tile([128, 1024], f16, tag="xt")
                nc.sync.dma_start(out=xt[:, :], in_=xT[:, 1024 * t:1024 * (t + 1)])
                xsq = xp.tile([128, 1024], f16, tag="xsq")
                nc.scalar.activation(xsq[:, :], xt[:, :], Act.Square)
                ps = ppA.tile([128, 512], f32, tag="acc")
                for h in (0, 1):
                    rhs = xt[:, :].rearrange("p (q two s) -> p two q s", two=2, s=S)[:, h]
                    sqh = xsq[:, :].rearrange("p (q two s) -> p two q s", two=2, s=S)[:, h]
                    o = ps[64 * h:64 * (h + 1), :].rearrange("m (q s) -> m q s", s=S)
                    nc.tensor.matmul(o, cbt_sb[:, :], rhs, start=True, stop=False)
                    nc.tensor.matmul(o, neghalf[:, :], sqh, start=False, stop=True)
                # g1 in layout2
                cm = wp.tile([128, 32], f32, tag="cm")
                ps3 = ps[:, :].rearrange("p (q s) -> p q s", s=S)
                nc.vector.tensor_reduce(cm[:, :], ps3, axis=mybir.AxisListType.X, op=Alu.max)
                a0 = wp.tile([128, 512], f32, tag="a0")
                cmb = cm[:, :].to_broadcast((128, 32, S))
                nc.vector.tensor_sub(a0[:, :].rearrange("p (q s) -> p q s", s=S), ps3, cmb)
                nc.scalar.activation(a0[:, :], a0[:, :], Act.Exp, scale=20.0)
                sg = wp.tile([128, 32], f32, tag="sg")
                nc.vector.tensor_reduce(sg[:, :], a0[:, :].rearrange("p (q s) -> p q s", s=S),
                                        axis=mybir.AxisListType.X, op=Alu.add)
                lg = wp.tile([128, 32], f32, tag="lg")
                nc.scalar.activation(lg[:, :], sg[:, :], Act.Ln)
                # glog20 = -(cm + lg/20 + log(1/16)/20)
                g20 = wp.tile([128, 32], f32, tag="g20")
                nc.vector.tensor_scalar(g20[:, :], lg[:, :], 1.0 / 20.0, LOG16_20,
                                        op0=Alu.mult, op1=Alu.add)
                nc.vector.tensor_add(g20[:, :], g20[:, :], cm[:, :])
                nc.vector.tensor_scalar_mul(g20[:, :], g20[:, :], -1.0)
                # M = PS + glog20  (still layout2)
                g20b = g20[:, :].to_broadcast((128, 32, S))
                m0 = wp.tile([128, 512], f32, tag="a0")
                nc.vector.tensor_add(m0[:, :].rearrange("p (q s) -> p q s", s=S), ps3, g20b)
                # transpose to layout1
                mt = ppB.tile([128, 512], f32, tag="mt")
                for c in range(4):
                    nc.tensor.transpose(mt[:, 128 * c:128 * (c + 1)],
                                        m0[:, 128 * c:128 * (c + 1)], ident[:, :])
                # f1 in layout1
                rm = wp.tile([128, 8], f32, tag="rm")
                mt3 = mt[:, :].rearrange("p (g k) -> p g k", k=K)
                nc.vector.tensor_reduce(rm[:, :], mt3, axis=mybir.AxisListType.X, op=Alu.max)
                a2 = wp.tile([128, 512], f32, tag="ps2")
                rmb = rm[:, :].to_broadcast((128, 8, K))
                nc.vector.tensor_sub(a2[:, :].rearrange("p (g k) -> p g k", k=K), mt3, rmb)
                Esl = E[:, 512 * t:512 * (t + 1)]
                nc.scalar.activation(Esl, a2[:, :], Act.Exp, scale=20.0)
                sf = wp.tile([128, 8], f32, tag="sf")
                nc.vector.tensor_reduce(sf[:, :], Esl.rearrange("p (g k) -> p g k", k=K),
                                        axis=mybir.AxisListType.X, op=Alu.add)
                nc.vector.tensor_scalar_mul(sf[:, :], sf[:, :], 1.0 / 64.0)
                u8 = wp.tile([128, 8], f32, tag="u8")
                nc.vector.reciprocal(u8[:, :], sf[:, :])
                u8b = u8[:, :].to_broadcast((128, 8, K))
                nc.vector.tensor_mul(Esl.rearrange("p (g k) -> p g k", k=K),
                                     Esl.rearrange("p (g k) -> p g k", k=K), u8b)

            # ---- 20 IPF iterations (unrolled; axon pipeline has no ctrl flow) ----
            groups = [list(range(g, min(g + 16, NT))) for g in range(0, NT, 16)]
            for _it in range(ITERS):
                for grp in groups:
                    scp = ppA.tile([128, 512], f32, tag="acc")
                    for v, t in enumerate(grp):
                        nc.tensor.matmul(scp[:, :], ones8p[:, 128 * v:128 * (v + 1)],
                                         E[:, 512 * t:512 * (t + 1)],
                                         start=(v == 0), stop=(v == len(grp) - 1))
                    vp = wp.tile([128, 512], f32, tag="vp")
                    nc.vector.reciprocal(vp[:, :], scp[:, :])
                    # process in sub-chunks of 8 so f-half interleaves finely
                    for s0 in range(0, len(grp), 8):
                        sub = grp[s0:s0 + 8]
                        for v, t in zip(range(s0, s0 + len(sub)), sub):
                            V = ppB.tile([128, 512], f32, tag="mt")
                            nc.tensor.matmul(V[:, :], bc16p[:, 128 * v:128 * (v + 1)],
                                             vp[:, :], start=True, stop=True)
                            Esl = E[:, 512 * t:512 * (t + 1)]
                            nc.vector.tensor_mul(Esl, Esl, V[:, :])
                        g0, gn = sub[0], len(sub)
                        Eg = E[:, 512 * g0:512 * (g0 + gn)].rearrange("p (g k) -> p g k", k=K)
                        sfb = wp.tile([128, 8 * gn], f32, tag="sfb")
                        nc.vector.tensor_reduce(sfb[:, :], Eg, axis=mybir.AxisListType.X, op=Alu.add)
                        nc.vector.tensor_scalar_mul(sfb[:, :], sfb[:, :], 1.0 / 64.0)
                        ub = wp.tile([128, 8 * gn], f32, tag="ub")
                        nc.vector.reciprocal(ub[:, :], sfb[:, :])
                        nc.vector.tensor_mul(Eg, Eg, ub[:, :].to_broadcast((128, 8 * gn, K)))

            # ---- final histogram = colsum_s(E), DMA out ----
            for t in range(NT):
                sc = ppA.tile([8, 512], f32, tag="acc")
                nc.tensor.matmul(sc[:, :], ones8[:, :], E[:, 512 * t:512 * (t + 1)],
                                 start=True, stop=True)
                hsb = wp.tile([8, 512], f16, tag="hsb")
                nc.scalar.copy(hsb[:, :], sc[:, :])
                nc.sync.dma_start(out=hist[:, 512 * t:512 * (t + 1)], in_=hsb[:, :])

    nc.finalize()
    return nc


def _ones8():
    a = np.zeros((128, 8), np.float32)
    for j in range(8):
        a[16 * j:16 * (j + 1), j] = 1.0
    return a


def _bc16():
    a = np.zeros((8, 128), np.float32)
    for j in range(8):
        a[j, 16 * j:16 * (j + 1)] = 16.0
    return a


def _host_prep(node_distributions, codebook):
    x16 = np.asarray(node_distributions).astype(np.float16)
    cb = np.asarray(codebook, dtype=np.float32)
    cbT = np.ascontiguousarray(cb.T).astype(np.float16)    # [128, 64]
    ones8, bc16 = _ones8(), _bc16()
    ident = np.eye(128, dtype=np.float32)
    in_maps = []
    for r in range(NCORES):
        xp = np.zeros((NPAD * S, D), np.float16)
        xp[:NPC * S] = x16[r * NPC:(r + 1) * NPC].reshape(NPC * S, D)
        xT = np.ascontiguousarray(xp.T)                    # [128, 40960] fp16
        in_maps.append({
            "xT": xT,
            "cbt": cbT,
            "ones8d": ones8,
            "bc16d": bc16,
            "identd": ident,
        })
    return in_maps


def _host_finish(hists, batch_idx, log_codebook_prior, num_graphs):
    """hists: list of [8, NT*512] per core -> pooled [B, K]."""
    bi = np.asarray(batch_idx).astype(np.int64)
    Bn = int(num_graphs)
    hn = np.empty((N, K), np.float32)
    for r, h in enumerate(hists):
        arr = h.reshape(8, NT, 4, 2, K)                    # [j, t, c, h, k]
        nodes = arr.transpose(1, 2, 0, 3, 4).reshape(NPAD, K)  # node = 64t+16c+2j+h
        hn[r * NPC:(r + 1) * NPC] = nodes[:NPC]
    hsum = hn.sum(-1)
    bad = ~np.isfinite(hsum) | (np.abs(hsum / 1024.0 - 1.0) > 1e-3) | (hn <= 0).any(-1)
    hn = hn / np.maximum(hsum, 1e-30)[:, None]
    global _last_bad_count
    _last_bad_count = int(bad.sum())
    if bad.any():      # exact host fallback for nodes the exp-domain device can't represent
        hn[bad] = _host_exact_par(np.where(bad)[0])
    sums = np.zeros((Bn, K), np.float32)
    np.add.at(sums, bi, hn)
    cnt = np.bincount(bi, minlength=Bn).astype(np.float32)
    prior = np.exp(log_codebook_prior - np.max(log_codebook_prior))
    prior = (prior / prior.sum()).astype(np.float32)
    return np.where(cnt[:, None] > 0, sums / np.maximum(cnt, 1.0)[:, None], prior[None, :])


_last_exec_ns = None
_last_bad_count = 0
_HOST_X = None
_HOST_CB = None


def _host_exact_par(idx):
    """Fork-parallel _host_exact; falls back to serial on any failure."""
    if len(idx) < 512:
        return _host_exact(idx)
    try:
        import os
        import multiprocessing as mp
        nw = max(1, min(8, (os.cpu_count() or 2) - 1))
        chunks = [c for c in np.array_split(idx, nw) if len(c)]
        ctx = mp.get_context("fork")
        with ctx.Pool(len(chunks)) as pool:
            parts = pool.map(_host_exact, chunks)
        return np.concatenate(parts)
    except Exception:
        return _host_exact(idx)


def _host_exact(idx):
    x = _HOST_X[idx].astype(np.float32)
    cb = _HOST_CB.astype(np.float32)
    C = np.maximum((x * x).sum(-1)[:, :, None] + (cb * cb).sum(-1)[None, None, :]
                   - 2 * np.einsum('nsd,kd->nsk', x, cb), 0).astype(np.float32)

    def lse(a, axis):
        m = np.max(a, axis=axis, keepdims=True)
        return np.squeeze(m, axis) + np.log(np.sum(np.exp(a - m), axis=axis))
    la = np.float32(-np.log(S))
    lb = np.full(K, -np.log(K), np.float32)
    f = np.zeros((len(idx), S), np.float32)
    g = np.zeros((len(idx), K), np.float32)
    for _ in range(21):
        g = -EPS * lse((f[:, :, None] - C) / EPS + la, 1)
        f = -EPS * lse((g[:, None, :] - C) / EPS + lb[None, None, :], 2)
    lp = (f[:, :, None] + g[:, None, :] - C) / EPS + la + lb[None, None, :]
    h = np.exp(lse(lp, 1))
    return (h / (h.sum(-1, keepdims=True) + 1e-12)).astype(np.float32)


def kernel(node_distributions, batch_idx, codebook, log_codebook_prior, num_graphs):
    global _HOST_X, _HOST_CB
    x = np.asarray(node_distributions, np.float32)
    cb = np.asarray(codebook, np.float32)
    lcp = np.asarray(log_codebook_prior, np.float32)
    _HOST_X, _HOST_CB = x, cb

    if not np.allclose(lcp, lcp.flat[0]):
        # general-prior fallback (harness uses zeros): exact host compute
        return _pool_host_full(x, np.asarray(batch_idx), cb, lcp, int(num_graphs))

    import time as _time
    in_maps = _host_prep(x, cb)
    nc = _build_bass()
    t0 = _time.time()
    hists = _dispatch_with_retry(nc, in_maps)
    global _last_exec_ns
    _last_exec_ns = int((_time.time() - t0) * 1e9)  # dispatch+transfer+exec wall
    return _host_finish(hists, batch_idx, lcp, num_graphs)


def _run_device(nc, in_maps):
    from concourse import bass2jax
    res_maps = bass2jax.run_bass_via_pjrt(nc, in_maps, n_cores=NCORES)
    return [np.ascontiguousarray(res_maps[r]["hist"], dtype=np.float16)
            for r in range(NCORES)]


def _dispatch_with_retry(nc, in_maps):
    """Run the device dispatch in a forked child. The axon tunnel stalls for
    ~60s on ~1 in 6 runs; a stalled attempt is killed at 30s and retried
    once (unbounded). Falls back to in-process dispatch on any fork issue.
    The parent never initializes the jax backend, so fork is safe."""
    import os, select, struct, time

    HCOUNT, HSHAPE = NCORES, (8, NT * 512)
    nbytes = HSHAPE[0] * HSHAPE[1] * 2                     # fp16 hist

    def _attempt(timeout_s):
        r, w = os.pipe()
        pid = os.fork()
        if pid == 0:                                       # child
            try:
                os.close(r)
                hists = _run_device(nc, in_maps)
                buf = b"".join(h.tobytes() for h in hists)
                os.write(w, struct.pack("<Q", len(buf)))
                view = memoryview(buf)
                while len(view):
                    n = os.write(w, view[:1 << 20])
                    view = view[n:]
                os.close(w)
            finally:
                os._exit(0)
        os.close(w)                                        # parent
        deadline = None if timeout_s is None else time.time() + timeout_s
        chunks, need = [], 8 + HCOUNT * nbytes
        got = 0
        try:
            while got < need:
                tmo = None if deadline is None else max(0.0, deadline - time.time())
                ready, _, _ = select.select([r], [], [], tmo)
                if not ready:
                    raise TimeoutError
                d = os.read(r, 1 << 20)
                if not d:
                    raise EOFError
                chunks.append(d)
                got += len(d)
        finally:
            os.close(r)
            if got < need:
                try:
                    os.kill(pid, 9)
                except OSError:
                    pass
            try:
                os.waitpid(pid, 0)
            except OSError:
                pass
        buf = b"".join(chunks)
        (blen,) = struct.unpack("<Q", buf[:8])
        assert blen == HCOUNT * nbytes
        flat = np.frombuffer(buf, np.float16, offset=8).reshape(HCOUNT, *HSHAPE)
        return [flat[c] for c in range(HCOUNT)]

    try:
        try:
            return _attempt(30.0)
        except (TimeoutError, EOFError, AssertionError, struct.error):
            return _attempt(None)
    except Exception:
        return _run_device(nc, in_maps)                    # last resort, in-process


def _pool_host_full(x, bi, cb, lcp, Bn):
    hn = np.concatenate([_host_exact(np.arange(i, min(i + 2000, x.shape[0])))
                         for i in range(0, x.shape[0], 2000)])
    sums = np.zeros((Bn, K), np.float32)
    np.add.at(sums, bi.astype(np.int64), hn)
    cnt = np.bincount(bi.astype(np.int64), minlength=Bn).astype(np.float32)
    prior = np.exp(lcp - lcp.max()); prior = (prior / prior.sum()).astype(np.float32)
    return np.where(cnt[:, None] > 0, sums / np.maximum(cnt, 1.0)[:, None], prior[None, :])

